# revision 2
# baseline (speedup 1.0000x reference)
"""Block-sparse attention backward pass on 8 TRN2 NeuronCores.

Sharding: head-parallel - 16 heads / 8 cores = 2 heads per core. The
block mask is shared by all heads, so every core runs the SAME program
(true SPMD); only the data shards differ. All dQ/dK/dV accumulation is
local to a head shard: no cross-core communication.

All heavy DVE work is chunk-granular (8 pairs per instruction) - the
DVE per-instruction overhead is ~220 ns, so per-pair DVE ops are fatal.

Math per chunk of 8 (i, j) block pairs:
  S_ij = q_i k_j^T                  (PE, row-group 0: stationary [qT;dOT])
  dA_ij = dO_i v_j^T                (PE, row-group 1, concurrent)
  U = exp(S * scale)                (ACT, one chunked inst)
  [l | rs] = group-rowsums of [U|W] (DVE, ONE bf16-out reduce -> 2x mode)
  r = 1/l  (approx)                 (DVE RECIPROCAL_APPROX_FAST, f32)
  rd2n = -rs o r^2                  (DVE, small)
  W = U o dA                        (DVE chunked, 1x: dA is f32 PSUM)
  [X|Y] = [U|W] o [rd2n|r]_bcast    (GPSIMD, one chunked broadcast mult)
  dS = X + Y                        (DVE chunked add, 2x all-bf16)
  dop = dO_i o r                    (GPSIMD broadcast, chunked, from dONp)
  dV^T_j += dop^T U                 (PE, shared bank cols 0-127)
  dK^T_j += (q*scale)^T dS          (PE, same bank cols 128-255)
  dS^T = dS^T @ I                   (PE NORMAL matmul vs identity: warm-
                                     rate and HAM-visible, f32 PSUM out)
  dQ^T_i += (k*scale)^T dS^T        (PE pass 2, j-major: stationary
                                     kns_j amortized; 16 i-accumulators
                                     live across 4 banks, 4 col-slots
                                     each, single bank-clear start=True)

dV/dK/dQ are produced transposed [64, N]; the host transposes back.

PSUM budget pass 1 (7/8 banks): s_ps 1x[128,1024]f32 (2) + da_ps
1x[128,1024]f32 (2) + dst 1x[128,1024]f32 (2) + dvk 1x[64,256]f32 (1).
The dvk bank holds dV^T (cols 0-127) and dK^T (cols 128-255): exactly
one start=True per j (first dV matmul) clears the bank; every other
matmul uses start=False and lands on has_written=0 regions (overwrite)
or accumulates - per-element semantics make the interleave correct.
"""

import sys

sys.path.insert(0, "/opt/trn_rl_repo")

import numpy as np
import ml_dtypes

import concourse.bass as bass
import concourse.mybir as mybir
import concourse.tile as tile
from concourse import bacc
from concourse.bass_utils import run_bass_kernel_spmd
from concourse.masks import make_identity

BF16 = mybir.dt.bfloat16
F32 = mybir.dt.float32
OP = mybir.AluOpType
ACTF = mybir.ActivationFunctionType

N, D, H, DK, BLK, T = 2048, 1024, 16, 64, 128, 16
NCORES, HPC = 8, 2  # heads per core
SCALE = float(1.0 / np.sqrt(DK))  # tau=1
CH = 8  # pairs per chunk

_BF = ml_dtypes.bfloat16


def _build(mask_key):
    """Build the SPMD program for one core (2 heads), specialized on the mask."""
    mask = np.array(mask_key, dtype=np.int64).reshape(T, T)
    act_per_j = [[i for i in range(T) if mask[i, j]] for j in range(T)]
    act_per_i = [[j for j in range(T) if mask[i, j]] for i in range(T)]
    npair = int(mask.sum())
    # flat pair stream, j-major; pidx[(i, j)] = position in stream
    pairs = [(i, j) for j in range(T) for i in act_per_j[j]]
    pidx = {p: n for n, p in enumerate(pairs)}
    # chunks of up to CH consecutive pairs
    chunks = [pairs[c:c + CH] for c in range(0, npair, CH)]

    nc = bacc.Bacc("TRN2", target_bir_lowering=False, debug=False)

    # per-head inputs
    qdo = [nc.dram_tensor(f"qdo{h}", [128, N], BF16, kind="ExternalInput")
           for h in range(HPC)]
    kv = [nc.dram_tensor(f"kv{h}", [128, N], BF16, kind="ExternalInput")
          for h in range(HPC)]
    qns = [nc.dram_tensor(f"qns{h}", [128, T * DK], BF16, kind="ExternalInput")
           for h in range(HPC)]
    kns = [nc.dram_tensor(f"kns{h}", [128, T * DK], BF16, kind="ExternalInput")
           for h in range(HPC)]
    donp = [nc.dram_tensor(f"donp{h}", [128, npair * DK], BF16,
                           kind="ExternalInput") for h in range(HPC)]

    # transposed outputs [head, d, N]; host transposes back
    dQo = nc.dram_tensor("dQo", [HPC, DK, N], F32, kind="ExternalOutput")
    dKo = nc.dram_tensor("dKo", [HPC, DK, N], F32, kind="ExternalOutput")
    dVo = nc.dram_tensor("dVo", [HPC, DK, N], F32, kind="ExternalOutput")

    with tile.TileContext(nc) as tc:
        with (
            tc.tile_pool(name="const", bufs=1) as constp,
            tc.tile_pool(name="inp", bufs=1) as inp,
            tc.tile_pool(name="dstore", bufs=1) as dstore,
            tc.tile_pool(name="uw", bufs=3) as uwp,
            tc.tile_pool(name="dsp", bufs=3) as dsp,
            tc.tile_pool(name="stat", bufs=3) as statp,
            tc.tile_pool(name="outsb", bufs=4) as outsb,
        ):
            ident = constp.tile([128, 128], BF16)
            make_identity(nc, ident[:])

            tqdo, tkv, tqns, tkns, tdonp = [], [], [], [], []
            for h in range(HPC):
                tqdo.append(inp.tile([128, N], BF16, name=f"tqdo{h}",
                                     tag=f"qdo{h}"))
                tkv.append(inp.tile([128, N], BF16, name=f"tkv{h}",
                                    tag=f"kv{h}"))
                tqns.append(inp.tile([128, T * DK], BF16, name=f"tqns{h}",
                                     tag=f"qns{h}"))
                tkns.append(inp.tile([128, T * DK], BF16, name=f"tkns{h}",
                                     tag=f"kns{h}"))
                tdonp.append(inp.tile([128, npair * DK], BF16,
                                      name=f"tdonp{h}", tag=f"donp{h}"))
                nc.sync.dma_start(tqdo[h][:], qdo[h][:])
                nc.sync.dma_start(tkv[h][:], kv[h][:])
                nc.sync.dma_start(tqns[h][:], qns[h][:])
                nc.sync.dma_start(tkns[h][:], kns[h][:])
                nc.sync.dma_start(tdonp[h][:], donp[h][:])

            # dS^T of every active pair, per head, bf16
            dstTs = [dstore.tile([128, npair * BLK], BF16, name=f"dstT{h}",
                                 tag=f"dstT{h}") for h in range(HPC)]

            seq = [(h, c) for h in range(HPC) for c in chunks]
            with (
                tc.tile_pool(name="ps_s", bufs=2, space="PSUM") as ps_s,
                tc.tile_pool(name="ps_da", bufs=1, space="PSUM") as ps_da,
                tc.tile_pool(name="ps_dst", bufs=1, space="PSUM") as ps_dst,
                tc.tile_pool(name="ps_dvk", bufs=1, space="PSUM") as ps_dvk,
            ):
                    dvk_st = {hh: [None, -1, 0] for hh in range(HPC)}

                    def flush_dvk(hh):
                        dvk, dvk_j, _ = dvk_st[hh]
                        if dvk is None:
                            return
                        sb = outsb.tile([DK, 2 * BLK], F32, name="dvksb",
                                        tag="dvk")
                        nc.scalar.copy(sb[:], dvk[:])
                        nc.sync.dma_start(
                            dVo[hh, :, dvk_j * BLK:(dvk_j + 1) * BLK],
                            sb[:, 0:BLK])
                        nc.sync.dma_start(
                            dKo[hh, :, dvk_j * BLK:(dvk_j + 1) * BLK],
                            sb[:, BLK:2 * BLK])
                        dvk_st[hh][0] = None

                    prev_h = 0
                    for h, chunk in seq:
                        if h != prev_h:
                            flush_dvk(prev_h)
                            prev_h = h
                        dstT = dstTs[h]
                        m = len(chunk)
                        p0 = pidx[chunk[0]]
                        s_ps = ps_s.tile([128, CH * BLK], F32, tag="s")
                        da_ps = ps_da.tile([128, CH * BLK], F32, tag="da")
                        dst_ps = ps_dst.tile([128, CH * BLK], BF16, tag="dst")
                        UW = uwp.tile([128, 2 * CH * BLK], BF16, tag="UW")
                        U = UW[:, :CH * BLK]
                        W = UW[:, CH * BLK:]
                        XY = dsp.tile([128, 2 * CH * BLK], BF16, tag="XY")
                        dS = dsp.tile([128, CH * BLK], BF16, tag="dS")
                        dop = dsp.tile([128, CH * DK], BF16, tag="dop")
                        # stb: [rd2n(0:CH) | r(CH:2CH)] f32, bcast operand
                        rsb = statp.tile([128, CH], BF16, tag="rsb")
                        stb = statp.tile([128, 2 * CH], F32, tag="stb")
                        lf = statp.tile([128, CH], F32, tag="lf")
                        rr = statp.tile([128, CH], F32, tag="rr")
                        rt = stb[:, CH:CH + m]

                        for x, (i, j) in enumerate(chunk):
                            cs = slice(x * BLK, (x + 1) * BLK)
                            nc.tensor.matmul(
                                s_ps[:, cs],
                                tqdo[h][0:DK, i * BLK:(i + 1) * BLK],
                                tkv[h][0:DK, j * BLK:(j + 1) * BLK],
                                start=True, stop=True,
                                tile_position=(0, 0))
                            nc.tensor.matmul(
                                da_ps[:, cs],
                                tqdo[h][DK:128, i * BLK:(i + 1) * BLK],
                                tkv[h][DK:128, j * BLK:(j + 1) * BLK],
                                start=True, stop=True,
                                tile_position=(DK, 0))

                        nc.scalar.activation(U[:, :m * BLK], s_ps[:, :m * BLK],
                                             ACTF.Exp, scale=SCALE)
                        nc.vector.tensor_reduce(
                            lf[:, 0:m],
                            U[:, :m * BLK].rearrange("p (g x) -> p g x", x=BLK),
                            axis=mybir.AxisListType.X, op=OP.add)
                        # W = U o dA, split per PSUM bank (a single DVE op
                        # crossing a bank boundary runs at half rate)
                        h1 = min(m, 4)
                        nc.vector.tensor_tensor(
                            W[:, :h1 * BLK], U[:, :h1 * BLK],
                            da_ps[:, :h1 * BLK], op=OP.mult)
                        if m > 4:
                            nc.vector.tensor_tensor(
                                W[:, 4 * BLK:m * BLK], U[:, 4 * BLK:m * BLK],
                                da_ps[:, 4 * BLK:m * BLK], op=OP.mult)
                        # rs: grouped reduce over W only, bf16 out
                        with nc.allow_low_precision("bf16 softmax stats"):
                            nc.vector.tensor_reduce(
                                rsb[:, 0:m],
                                W[:, :m * BLK].rearrange(
                                    "p (g x) -> p g x", x=BLK),
                                axis=mybir.AxisListType.X, op=OP.add)
                        nc.vector.reciprocal_approx_fast(out=rt,
                                                         in_=lf[:, 0:m])
                        nc.vector.tensor_tensor(rr[:, 0:m], rt, rt,
                                                op=OP.mult)
                        # rd2n = -rs * r^2
                        nc.vector.scalar_tensor_tensor(
                            out=stb[:, 0:m], in0=rr[:, 0:m], scalar=-1.0,
                            in1=rsb[:, 0:m], op0=OP.mult, op1=OP.mult)
                        # dop = dOblk o r first: the dV matmuls need it
                        nc.gpsimd.tensor_tensor(
                            dop[:, :m * DK].rearrange("p (g x) -> p g x", x=DK),
                            tdonp[h][:, p0 * DK:(p0 + m) * DK].rearrange(
                                "p (g x) -> p g x", x=DK),
                            rt[:, :, None].broadcast_to([128, m, DK]),
                            op=OP.mult)
                        # [X|Y] = [U|W] o [rd2n|r]  (one broadcast mult)
                        nc.gpsimd.tensor_tensor(
                            XY[:].rearrange("p (g x) -> p g x", x=BLK),
                            UW[:].rearrange("p (g x) -> p g x", x=BLK),
                            stb[:, 0:2 * CH][:, :, None]
                            .broadcast_to([128, 2 * CH, BLK]),
                            op=OP.mult)
                        nc.vector.tensor_tensor(
                            dS[:, :m * BLK], XY[:, :m * BLK],
                            XY[:, CH * BLK:CH * BLK + m * BLK], op=OP.add)

                        for x, (i, j) in enumerate(chunk):
                            cs = slice(x * BLK, (x + 1) * BLK)
                            if j != dvk_st[h][1]:
                                flush_dvk(h)
                                dvk_st[h] = [
                                    ps_dvk.tile([DK, 2 * BLK], F32,
                                                name="dvkps", tag="dvk"),
                                    j, 0]
                            dvk, _, dvk_n = dvk_st[h]
                            npair_j = len(act_per_j[j])
                            first = dvk_n == 0
                            last = dvk_n == npair_j - 1
                            # dV^T_j += dop_x^T U_x   (bank cols 0-127)
                            nc.tensor.matmul(
                                dvk[:, 0:BLK],
                                dop[:, x * DK:(x + 1) * DK],
                                U[:, cs],
                                start=first, stop=last,
                                skip_group_check=True)
                            # dK^T_j += qns_i^T dS_x  (bank cols 128-255)
                            nc.tensor.matmul(
                                dvk[:, BLK:2 * BLK],
                                tqns[h][:, i * DK:(i + 1) * DK],
                                dS[:, cs],
                                start=False, stop=last,
                                skip_group_check=True)
                            dvk_st[h][2] += 1
                            nc.tensor.transpose(dst_ps[:, cs], dS[:, cs],
                                                ident[:])
                        nc.scalar.copy(dstT[:, p0 * BLK:(p0 + m) * BLK],
                                       dst_ps[:, :m * BLK])
                    for hh in range(HPC):
                        flush_dvk(hh)

            for h in range(HPC):
                dstT = dstTs[h]
                # pass 2: dQ^T_i = sum_j kns_j^T dS^T_ij, j-major so the
                # kns_j stationary is shared by consecutive matmuls. 16
                # i-accumulators live across 4 banks x 4 column slots.
                with tc.tile_pool(name="ps_dq", bufs=1, space="PSUM") as ps_dq:
                    NB = 4  # i-accumulator banks
                    bank_pairs = [[] for _ in range(NB)]
                    for j in range(T):
                        for i in act_per_j[j]:
                            bank_pairs[i // 4].append((i, j))
                    dq_tiles = [ps_dq.tile([DK, 4 * BLK], F32,
                                           name=f"dqps{t}", tag=f"dq{t}")
                                for t in range(NB)]
                    for j in range(T):
                        for i in act_per_j[j]:
                            t, s = i // 4, i % 4
                            nc.tensor.matmul(
                                dq_tiles[t][:, s * BLK:(s + 1) * BLK],
                                tkns[h][:, j * DK:(j + 1) * DK],
                                dstT[:, pidx[(i, j)] * BLK:
                                     (pidx[(i, j)] + 1) * BLK],
                                start=(bank_pairs[t][0] == (i, j)),
                                stop=(bank_pairs[t][-1] == (i, j)),
                                skip_group_check=True)
                    for t in range(NB):
                        if not bank_pairs[t]:
                            continue
                        sb = outsb.tile([DK, 4 * BLK], F32, name="dqsb",
                                        tag="dq")
                        nc.scalar.copy(sb[:], dq_tiles[t][:])
                        for s in range(4):
                            i = t * 4 + s
                            if act_per_i[i]:
                                nc.sync.dma_start(
                                    dQo[h, :, i * BLK:(i + 1) * BLK],
                                    sb[:, s * BLK:(s + 1) * BLK])
    nc.compile()
    return nc, npair, pairs


_prog_cache = {}


def _get_prog(mask):
    key = tuple(int(x) for x in np.asarray(mask).astype(np.int64).ravel())
    if key not in _prog_cache:
        _prog_cache[key] = _build(key)
    return _prog_cache[key]


def kernel(q, k, v, dO, block_sparse_mask, _trace=False):
    q = np.ascontiguousarray(np.asarray(q, dtype=np.float32))
    k = np.ascontiguousarray(np.asarray(k, dtype=np.float32))
    v = np.ascontiguousarray(np.asarray(v, dtype=np.float32))
    dO = np.ascontiguousarray(np.asarray(dO, dtype=np.float32))
    mask = np.asarray(block_sparse_mask)

    nc, npair, pairs = _get_prog(mask)

    def tlay(x, g):  # head g of (1,N,D) -> [64, N] transposed bf16
        return np.ascontiguousarray(
            x[0, :, g * DK:(g + 1) * DK].T).astype(_BF)

    def nlay(x, g, scale):  # head g natural -> [128, T*DK]
        y = (x[0, :, g * DK:(g + 1) * DK] * scale).reshape(T, BLK, DK)
        return np.ascontiguousarray(
            y.transpose(1, 0, 2).reshape(BLK, T * DK)).astype(_BF)

    in_maps = []
    for c in range(NCORES):
        im = {}
        for h in range(HPC):
            g = c * HPC + h
            im[f"qdo{h}"] = np.ascontiguousarray(
                np.concatenate([tlay(q, g), tlay(dO, g)], axis=0))
            im[f"kv{h}"] = np.ascontiguousarray(
                np.concatenate([tlay(k, g), tlay(v, g)], axis=0))
            im[f"qns{h}"] = nlay(q, g, SCALE)
            im[f"kns{h}"] = nlay(k, g, SCALE)
            don = nlay(dO, g, 1.0).reshape(BLK, T, DK)
            im[f"donp{h}"] = np.ascontiguousarray(
                don[:, [i for (i, j) in pairs], :].reshape(BLK, npair * DK))
        in_maps.append(im)

    res = run_bass_kernel_spmd(nc, in_maps, list(range(NCORES)), trace=_trace)
    if _trace:
        kernel.last_exec_time_ns = res.exec_time_ns
        kernel.last_res = res

    m64 = np.asarray(mask).astype(np.int64)
    empty_i = [i for i in range(T) if not m64[i, :].any()]
    empty_j = [j for j in range(T) if not m64[:, j].any()]

    dQ = np.empty((1, N, D), np.float32)
    dK = np.empty((1, N, D), np.float32)
    dV = np.empty((1, N, D), np.float32)
    for c in range(NCORES):
        r = res.results[c]
        for h in range(HPC):
            g = c * HPC + h
            dQ[0, :, g * DK:(g + 1) * DK] = r["dQo"][h].T
            dK[0, :, g * DK:(g + 1) * DK] = r["dKo"][h].T
            dV[0, :, g * DK:(g + 1) * DK] = r["dVo"][h].T
    for i in empty_i:
        dQ[0, i * BLK:(i + 1) * BLK, :] = 0.0
    for j in empty_j:
        dK[0, j * BLK:(j + 1) * BLK, :] = 0.0
        dV[0, j * BLK:(j + 1) * BLK, :] = 0.0
    return dQ, dK, dV



# revision 12
# speedup vs baseline: 1.0508x; 1.0508x over previous
"""Block-sparse attention backward pass on 8 TRN2 NeuronCores (v11).

Sharding: head-parallel - 16 heads / 8 cores = 2 heads per core; every
core runs the same program specialized on the (replicated) block mask.

Math per (i, j) active block pair (local per-block softmax):
  S = q_i k_j^T s          U = exp(S)        l = rowsum(U)   rt = 1/l
  dA = dO_i v_j^T          rs = rowsum(U o dA)
  G = dA*rt - (rs*rt^2)    dS = U o G        (dS == Pn o (dA - rd))
  dV_j += Pn^T dO_i = U^T (dO_i o rt)        (dop = dO o rt)
  dK_j += dS^T (q_i s)     dQ_i += dS (k_j s)

Engine plan per chunk of 8 pairs (j-major pair stream):
  PE-A   : S, dA matmuls (concurrent row groups h0/h64), f32 PSUM
  ACT    : U = exp(s_ps) chunked; dAb = copy(da_ps) -> bf16 SBUF
  DVE    : 8x tensor_scalar(U)+accum -> l (4x mode); reciprocal -> rt;
           8x scalar_tensor_tensor(dAb, U)+accum -> rs (1x);
           rt2/rdr2 smalls; 8x tensor_scalar(dAb; rt, rdr2) -> G (4x)
  Pool   : dop broadcast-mult (chunked); dS = U o G (chunked TT)
  DMA    : 8x XBAR transpose dS -> dS^T (SBUF->SBUF, free engines)
  PE-B   : dV^T_j += dop_x^T U_x   (A-form: [64,128] out)
           dK_j   += dS_x^T qns_i  (B-form: stationary dS, 64-col moving)
           dQ_i   += dST_x^T kns_j (B-form: 16 live accumulators)
  PE-B for chunk c is emitted after PE-A of chunk c+LAG (software
  pipeline) so the PE never waits on the long DVE chain.

PSUM: s_ps 2 banks + da_ps 2 banks (single-buffered; freed early by
exp/dAb) + dvk pool 2 + dq accumulator 2 = 8.

Outputs: dVo [H,64,N] transposed; dKo/dQo [H,N,64] natural.
"""

import sys

sys.path.insert(0, "/opt/trn_rl_repo")

import numpy as np
import ml_dtypes

import concourse.bass as bass
import concourse.mybir as mybir
import concourse.tile as tile
from concourse import bacc
from concourse.bass_utils import run_bass_kernel_spmd
from concourse.masks import make_identity

BF16 = mybir.dt.bfloat16
F32 = mybir.dt.float32
OP = mybir.AluOpType
ACTF = mybir.ActivationFunctionType

N, D, H, DK, BLK, T = 2048, 1024, 16, 64, 128, 16
NCORES, HPC = 8, 2
SCALE = float(1.0 / np.sqrt(DK))  # tau=1
CH = 8   # pairs per chunk
LAG = 2  # software-pipeline distance between PE-A and PE-B

_BF = ml_dtypes.bfloat16


def _build(mask_key):
    mask = np.array(mask_key, dtype=np.int64).reshape(T, T)
    act_per_j = [[i for i in range(T) if mask[i, j]] for j in range(T)]
    act_per_i = [[j for j in range(T) if mask[i, j]] for i in range(T)]
    npair = int(mask.sum())
    pairs = [(i, j) for j in range(T) for i in act_per_j[j]]
    chunks = [pairs[c:c + CH] for c in range(0, npair, CH)]
    nch = len(chunks)
    # stream index of the last occurrence of each j / i
    last_of_j = {}
    last_of_i = {}
    for n, (i, j) in enumerate(pairs):
        last_of_j[j] = n
        last_of_i[i] = n
    # dq accumulator: slots 0-7 in PSUM bank A, 8-15 in bank B. Exactly one
    # start=True per (head, bank) - start resets has_written for the whole
    # bank; later slot-first matmuls overwrite via has_written=0.
    bank_ns = {0: [], 1: []}
    for n, (i, j) in enumerate(pairs):
        bank_ns[i // 8].append(n)
    dq_first = {b: ns[0] for b, ns in bank_ns.items() if ns}
    dq_last = {b: ns[-1] for b, ns in bank_ns.items() if ns}

    nc = bacc.Bacc("TRN2", target_bir_lowering=False, debug=False)

    qdo = [nc.dram_tensor(f"qdo{h}", [128, N], BF16, kind="ExternalInput")
           for h in range(HPC)]
    kv = [nc.dram_tensor(f"kv{h}", [128, N], BF16, kind="ExternalInput")
          for h in range(HPC)]
    qns = [nc.dram_tensor(f"qns{h}", [128, T * DK], BF16, kind="ExternalInput")
           for h in range(HPC)]
    kns = [nc.dram_tensor(f"kns{h}", [128, T * DK], BF16, kind="ExternalInput")
           for h in range(HPC)]
    donp = [nc.dram_tensor(f"donp{h}", [128, npair * DK], BF16,
                           kind="ExternalInput") for h in range(HPC)]

    dVo = nc.dram_tensor("dVo", [HPC, DK, N], F32, kind="ExternalOutput")
    dKo = nc.dram_tensor("dKo", [HPC, N, DK], F32, kind="ExternalOutput")
    dQo = nc.dram_tensor("dQo", [HPC, N, DK], F32, kind="ExternalOutput")

    with tile.TileContext(nc) as tc:
        with (
            tc.tile_pool(name="const", bufs=1) as constp,
            tc.tile_pool(name="inp", bufs=1) as inp,
            tc.tile_pool(name="uwp", bufs=4) as uwp,        # [U|W] tiles
            tc.tile_pool(name="dabp", bufs=2) as dabp,      # dAb tiles
            tc.tile_pool(name="xyp", bufs=2) as xyp,        # XY tiles
            tc.tile_pool(name="dsp", bufs=4) as dsp,        # dS tiles
            tc.tile_pool(name="dstp", bufs=4) as dstp,      # dS^T tiles
            tc.tile_pool(name="dopp", bufs=4) as dopp,      # dop tiles
            tc.tile_pool(name="statp", bufs=4) as statp,
            tc.tile_pool(name="outsb", bufs=4) as outsb,
            tc.tile_pool(name="ps_s", bufs=1, space="PSUM") as ps_s,
            tc.tile_pool(name="ps_da", bufs=1, space="PSUM") as ps_da,
            tc.tile_pool(name="ps_dst", bufs=1, space="PSUM") as ps_dst,
            tc.tile_pool(name="ps_dvk", bufs=1, space="PSUM") as ps_dvk,
            tc.tile_pool(name="ps_dq", bufs=1, space="PSUM") as ps_dq,
        ):
            ident = constp.tile([128, 128], BF16)
            make_identity(nc, ident[:])
            tqdo, tkv, tqns, tkns, tdonp = [], [], [], [], []
            for h in range(HPC):
                tqdo.append(inp.tile([128, N], BF16, name=f"tqdo{h}",
                                     tag=f"qdo{h}"))
                tkv.append(inp.tile([128, N], BF16, name=f"tkv{h}",
                                    tag=f"kv{h}"))
                tqns.append(inp.tile([128, T * DK], BF16, name=f"tqns{h}",
                                     tag=f"qns{h}"))
                tkns.append(inp.tile([128, T * DK], BF16, name=f"tkns{h}",
                                     tag=f"kns{h}"))
                tdonp.append(inp.tile([128, npair * DK], BF16,
                                      name=f"tdonp{h}", tag=f"donp{h}"))
                nc.sync.dma_start(tqdo[h][:], qdo[h][:])
                nc.sync.dma_start(tkv[h][:], kv[h][:])
                nc.sync.dma_start(tqns[h][:], qns[h][:])
                nc.sync.dma_start(tkns[h][:], kns[h][:])
                nc.sync.dma_start(tdonp[h][:], donp[h][:])

            # dQ accumulator: 16 slots of [128, 64] f32 = 2 banks, shared
            # across heads (tile deps serialize the head handoff).
            dq_tile = ps_dq.tile([128, T * DK], F32, name="dqacc", tag="dq")

            # per-chunk SBUF tiles of the in-flight window
            win = {}

            dvk_st = [None, -1, 0]  # [tile, j, nacc]

            def emit_A(h, c):
                chunk = chunks[c]
                m = len(chunk)
                s_ps = ps_s.tile([128, CH * BLK], F32, tag="s")
                da_ps = ps_da.tile([128, CH * BLK], F32, tag="da")
                dst_ps = ps_dst.tile([128, CH * BLK], BF16, tag="dst")
                UW = uwp.tile([128, 2 * CH * BLK], BF16, tag="UW")
                U = UW[:, :CH * BLK]
                W = UW[:, CH * BLK:]
                dAb = dabp.tile([128, CH * BLK], BF16, tag="dAb")
                XY = xyp.tile([128, 2 * CH * BLK], BF16, tag="XY")
                dS = dsp.tile([128, CH * BLK], BF16, tag="dS")
                dST = dstp.tile([128, CH * BLK], BF16, tag="dST")
                dop = dopp.tile([128, CH * DK], BF16, tag="dop")
                # stb: [rd2n(0:CH) | rt(CH:2CH)] f32 broadcast operand
                stb = statp.tile([128, 2 * CH], F32, tag="stb")
                lf = statp.tile([128, CH], F32, tag="lf")
                rs = statp.tile([128, CH], F32, tag="rs")
                rr = statp.tile([128, CH], F32, tag="rr")
                rt = stb[:, CH:2 * CH]

                for x, (i, j) in enumerate(chunk):
                    cs = slice(x * BLK, (x + 1) * BLK)
                    nc.tensor.matmul(
                        s_ps[:, cs],
                        tqdo[h][0:DK, i * BLK:(i + 1) * BLK],
                        tkv[h][0:DK, j * BLK:(j + 1) * BLK],
                        start=True, stop=True, tile_position=(0, 0))
                    nc.tensor.matmul(
                        da_ps[:, cs],
                        tqdo[h][DK:128, i * BLK:(i + 1) * BLK],
                        tkv[h][DK:128, j * BLK:(j + 1) * BLK],
                        start=True, stop=True, tile_position=(DK, 0))

                nc.scalar.activation(U[:, :m * BLK], s_ps[:, :m * BLK],
                                     ACTF.Exp, scale=SCALE)
                nc.scalar.copy(dAb[:, :m * BLK], da_ps[:, :m * BLK])

                # l = grouped rowsum(U); rt = 1/l
                nc.vector.tensor_reduce(
                    lf[:, 0:m],
                    U[:, :m * BLK].rearrange("p (g x) -> p g x", x=BLK),
                    axis=mybir.AxisListType.X, op=OP.add)
                nc.vector.reciprocal_approx_fast(out=rt[:, 0:m],
                                                 in_=lf[:, 0:m])

                # dop = dO o rt (chunked broadcast on Pool)
                p0 = c * CH
                nc.gpsimd.tensor_tensor(
                    dop[:, :m * DK].rearrange("p (g x) -> p g x", x=DK),
                    tdonp[h][:, p0 * DK:(p0 + m) * DK].rearrange(
                        "p (g x) -> p g x", x=DK),
                    rt[:, 0:m][:, :, None].broadcast_to([128, m, DK]),
                    op=OP.mult)

                # W = U o dAb (2x all-bf16 SBUF); rs = grouped rowsum(W)
                nc.vector.tensor_tensor(W[:, :m * BLK], U[:, :m * BLK],
                                        dAb[:, :m * BLK], op=OP.mult)
                nc.vector.tensor_reduce(
                    rs[:, 0:m],
                    W[:, :m * BLK].rearrange("p (g x) -> p g x", x=BLK),
                    axis=mybir.AxisListType.X, op=OP.add)
                nc.vector.tensor_tensor(rr[:, 0:m], rt[:, 0:m], rt[:, 0:m],
                                        op=OP.mult)
                # rd2n = -rs * rt^2
                nc.vector.scalar_tensor_tensor(
                    out=stb[:, 0:m], in0=rr[:, 0:m], scalar=-1.0,
                    in1=rs[:, 0:m], op0=OP.mult, op1=OP.mult)

                # [X|Y] = [U|W] o [rd2n|rt]  (one chunked broadcast on Pool)
                nc.gpsimd.tensor_tensor(
                    XY[:].rearrange("p (g x) -> p g x", x=BLK),
                    UW[:].rearrange("p (g x) -> p g x", x=BLK),
                    stb[:, 0:2 * CH][:, :, None]
                    .broadcast_to([128, 2 * CH, BLK]),
                    op=OP.mult)
                # dS = X + Y (2x)
                nc.vector.tensor_tensor(
                    dS[:, :m * BLK], XY[:, :m * BLK],
                    XY[:, CH * BLK:CH * BLK + m * BLK], op=OP.add)

                # dS^T per pair via PE transpose; copy to SBUF on ACT
                for x in range(m):
                    cs = slice(x * BLK, (x + 1) * BLK)
                    nc.tensor.transpose(dst_ps[:, cs], dS[:, cs], ident[:])
                nc.scalar.copy(dST[:, :m * BLK], dst_ps[:, :m * BLK])

                win[(h, c)] = (U, dS, dST, dop)

            def flush_dvk(h):
                dvk, j, _ = dvk_st
                if dvk is None:
                    return
                sb = outsb.tile([128, 192], F32, tag="dvksb")
                nc.scalar.copy(sb[:], dvk[:, 0:192])
                nc.sync.dma_start(dVo[h, :, j * BLK:(j + 1) * BLK],
                                  sb[0:DK, 64:192])
                nc.sync.dma_start(dKo[h, j * BLK:(j + 1) * BLK, :],
                                  sb[:, 0:DK])
                dvk_st[0] = None

            def emit_B(h, c):
                chunk = chunks[c]
                U, dS, dST, dop = win.pop((h, c))
                for x, (i, j) in enumerate(chunk):
                    n = c * CH + x
                    cs = slice(x * BLK, (x + 1) * BLK)
                    if j != dvk_st[1] or dvk_st[0] is None:
                        flush_dvk(h)
                        # bank-sized tile: each buf must own a full PSUM
                        # bank (start=True resets has_written bank-wide)
                        dvk_st[0] = ps_dvk.tile([128, 512], F32,
                                                name="dvkps", tag="dvk")
                        dvk_st[1] = j
                        dvk_st[2] = 0
                    dvk = dvk_st[0]
                    npair_j = len(act_per_j[j])
                    first = dvk_st[2] == 0
                    last = dvk_st[2] == npair_j - 1
                    # dK_j += dS_x^T qns_i  ([128,64] out). Only this first
                    # matmul carries start=True: it spans all 128
                    # partitions, so the bank-wide has_written clear covers
                    # the dV region too; dV's first write then lands on
                    # has_written=0 (overwrite).
                    nc.tensor.matmul(
                        dvk[:, 0:DK],
                        dS[:, cs],
                        tqns[h][:, i * DK:(i + 1) * DK],
                        start=first, stop=last, skip_group_check=True)
                    # dV^T_j += dop_x^T U_x  ([64,128] out)
                    nc.tensor.matmul(
                        dvk[0:DK, 64:192],
                        dop[:, x * DK:(x + 1) * DK],
                        U[:, cs],
                        start=False, stop=last, skip_group_check=True)
                    dvk_st[2] += 1
                    # dQ_i += dST_x^T kns_j  (slot i of dq_tile)
                    nc.tensor.matmul(
                        dq_tile[:, i * DK:(i + 1) * DK],
                        dST[:, cs],
                        tkns[h][:, j * DK:(j + 1) * DK],
                        start=(dq_first[i // 8] == n),
                        stop=(dq_last[i // 8] == n),
                        skip_group_check=True)
                    if n == last_of_j[j]:
                        flush_dvk(h)

            def flush_dq(h):
                sbq = outsb.tile([128, T * DK], F32, tag="dqsb")
                nc.scalar.copy(sbq[:], dq_tile[:])
                nc.sync.dma_start(
                    dQo[h].rearrange("(t p) d -> p t d", p=BLK),
                    sbq[:].rearrange("p (t d) -> p t d", d=DK))

            # flat (h, c) stream; PE-B lags PE-A by LAG chunks, crossing
            # head boundaries so the PE never drains between heads.
            seq = [(h, c) for h in range(HPC) for c in range(nch)]
            for idx, (h, c) in enumerate(seq):
                emit_A(h, c)
                if idx >= LAG:
                    hb, cb = seq[idx - LAG]
                    emit_B(hb, cb)
                    if cb == nch - 1:
                        flush_dq(hb)
            for idx in range(len(seq) - LAG, len(seq)):
                hb, cb = seq[idx]
                emit_B(hb, cb)
                if cb == nch - 1:
                    flush_dq(hb)

    nc.compile()
    return nc, npair, pairs


_prog_cache = {}


def _get_prog(mask):
    key = tuple(int(x) for x in np.asarray(mask).astype(np.int64).ravel())
    if key not in _prog_cache:
        _prog_cache[key] = _build(key)
    return _prog_cache[key]


def kernel(q, k, v, dO, block_sparse_mask, _trace=False):
    q = np.ascontiguousarray(np.asarray(q, dtype=np.float32))
    k = np.ascontiguousarray(np.asarray(k, dtype=np.float32))
    v = np.ascontiguousarray(np.asarray(v, dtype=np.float32))
    dO = np.ascontiguousarray(np.asarray(dO, dtype=np.float32))
    mask = np.asarray(block_sparse_mask)

    nc, npair, pairs = _get_prog(mask)

    def tlay(x, g):  # head g of (1,N,D) -> [64, N] transposed bf16
        return np.ascontiguousarray(
            x[0, :, g * DK:(g + 1) * DK].T).astype(_BF)

    def nlay(x, g, scale):  # head g natural -> [128, T*DK]
        y = (x[0, :, g * DK:(g + 1) * DK] * scale).reshape(T, BLK, DK)
        return np.ascontiguousarray(
            y.transpose(1, 0, 2).reshape(BLK, T * DK)).astype(_BF)

    in_maps = []
    for c in range(NCORES):
        im = {}
        for h in range(HPC):
            g = c * HPC + h
            im[f"qdo{h}"] = np.ascontiguousarray(
                np.concatenate([tlay(q, g), tlay(dO, g)], axis=0))
            im[f"kv{h}"] = np.ascontiguousarray(
                np.concatenate([tlay(k, g), tlay(v, g)], axis=0))
            im[f"qns{h}"] = nlay(q, g, SCALE)
            im[f"kns{h}"] = nlay(k, g, SCALE)
            don = nlay(dO, g, 1.0).reshape(BLK, T, DK)
            im[f"donp{h}"] = np.ascontiguousarray(
                don[:, [i for (i, j) in pairs], :].reshape(BLK, npair * DK))
        in_maps.append(im)

    res = run_bass_kernel_spmd(nc, in_maps, list(range(NCORES)), trace=_trace)
    if _trace:
        kernel.last_exec_time_ns = res.exec_time_ns
        kernel.last_res = res

    m64 = np.asarray(mask).astype(np.int64)
    empty_i = [i for i in range(T) if not m64[i, :].any()]
    empty_j = [j for j in range(T) if not m64[:, j].any()]

    dQ = np.empty((1, N, D), np.float32)
    dK = np.empty((1, N, D), np.float32)
    dV = np.empty((1, N, D), np.float32)
    for c in range(NCORES):
        r = res.results[c]
        for h in range(HPC):
            g = c * HPC + h
            dQ[0, :, g * DK:(g + 1) * DK] = r["dQo"][h]
            dK[0, :, g * DK:(g + 1) * DK] = r["dKo"][h]
            dV[0, :, g * DK:(g + 1) * DK] = r["dVo"][h].T
    for i in empty_i:
        dQ[0, i * BLK:(i + 1) * BLK, :] = 0.0
    for j in empty_j:
        dK[0, j * BLK:(j + 1) * BLK, :] = 0.0
        dV[0, j * BLK:(j + 1) * BLK, :] = 0.0
    return dQ, dK, dV
